# revision 21
# baseline (speedup 1.0000x reference)
"""Trainium2 Bass kernel for CompactPiecewiseLinearEmbeddings.

out[n, f*8+d] = sum_b h[n,f,b] * W[f,b,d] + b[f,d]
h = piecewise-linear encoding of x[n,f] over per-feature bins
    (first bin clamp_max(1), middle clamp(0,1), last clamp_min(0)).

Strategy (per core; data-parallel over N across 8 cores):
 - Host passes x pre-transposed (xT [F, NS]) and gathers the
   [F*D, NS]-major output with a host-side transpose (both are part of
   shard/unshard).
 - Uniform row space r = f*48 + k over all features/bins (k=0 is the
   bin0 special row, k=1..46 middle bins, k=47 the bin47 special row):
   96 tiles of 128 rows per n-chunk; 16-feature output group = exactly
   6 tiles.
 - Per tile: PE broadcast matmul (signed selector) puts +-x on the 128
   rows; pass1 builds the relu basis (ACT relu(scale*bc+bias), or DVE
   subtract/max, interleaved 2:1 for balance); pass2 clamps to [0, 1]
   on DVE (const-only min for the ACT route, mult/min for the DVE
   route); PE block matmul accumulates 6 tiles into the group's
   [128, CH] PSUM output.
 - ACT or DVE evacuates + adds bias; DMA straight to HBM in [fd, n]
   layout.
"""
import numpy as np
import ml_dtypes

from concourse import bacc, mybir
from concourse.tile import TileContext
from concourse.bass_utils import run_bass_kernel_spmd

N, F, B, D = 16384, 256, 48, 8
NCORES = 8
NS = N // NCORES          # 2048 rows per core
CH = 512                  # n-chunk (PSUM bank = 512 f32 out cols per MM)
NCH = NS // CH            # 4
NGRP = F // 16            # 16 groups of 16 features
NT = F * B // 128         # 96 row-tiles
TPG = NT // NGRP          # 6 tiles per group

_cache = {}


def build_nc():
    nc = bacc.Bacc("TRN2")
    f32, f16, bf16 = mybir.dt.float32, mybir.dt.float16, mybir.dt.bfloat16

    xT_ext = nc.declare_dram_parameter("xT", [F, NS], f32, isOutput=False)
    sel_ext = nc.declare_dram_parameter("sel", [128, NT * 128], f16, isOutput=False)
    wpack_ext = nc.declare_dram_parameter("wpack", [128, NT * 128], bf16, isOutput=False)
    scaleT_ext = nc.declare_dram_parameter("scaleT", [128, NT], f32, isOutput=False)
    biasT_ext = nc.declare_dram_parameter("biasT", [128, NT], f32, isOutput=False)
    eT_ext = nc.declare_dram_parameter("eT", [128, NT], f32, isOutput=False)
    wvD_ext = nc.declare_dram_parameter("wvD", [128, NT], f32, isOutput=False)
    obias_ext = nc.declare_dram_parameter("obias", [128, NGRP], f32, isOutput=False)
    out_ext = nc.declare_dram_parameter("out", [F * D, NS], f32, isOutput=True)

    with TileContext(nc) as tc:
        with (
            tc.tile_pool(name="const", bufs=1) as cpool,
            tc.tile_pool(name="hbuf", bufs=9) as hpool,
            tc.tile_pool(name="h2buf", bufs=9) as h2pool,
            tc.tile_pool(name="osb", bufs=6) as opool,
            tc.tile_pool(name="bc", bufs=4, space="PSUM") as bcpool,
            tc.tile_pool(name="oc", bufs=3, space="PSUM") as ocpool,
        ):
            # ---- load constants ----
            sel = cpool.tile([128, NT * 128], f16)
            wpack = cpool.tile([128, NT * 128], bf16)
            scaleT = cpool.tile([128, NT], f32)
            biasT = cpool.tile([128, NT], f32)
            eT = cpool.tile([128, NT], f32)
            wvD = cpool.tile([128, NT], f32)
            obias = cpool.tile([128, NGRP], f32)
            for t, e in [(sel, sel_ext), (wpack, wpack_ext), (scaleT, scaleT_ext),
                         (biasT, biasT_ext), (eT, eT_ext), (wvD, wvD_ext), (obias, obias_ext)]:
                nc.sync.dma_start(out=t[:], in_=e[:])

            # ---- load xT (f32) and cast to f16 rhs tiles ----
            xsb = cpool.tile([128, 2 * NS], f32)
            for ft in range(2):
                nc.sync.dma_start(out=xsb[:, ft * NS:(ft + 1) * NS],
                                  in_=xT_ext[ft * 128:(ft + 1) * 128, :])
            xT = [cpool.tile([128, NS], f16, tag=f"xT{t}", name=f"xT{t}")
                  for t in range(2)]
            for ft in range(2):
                nc.scalar.activation(xT[ft][:], xsb[:, ft * NS:(ft + 1) * NS],
                                     mybir.ActivationFunctionType.Copy)

            # ---- main loop ----
            for c in range(NCH):
                for g in range(NGRP):
                    oc = ocpool.tile([128, CH], f32, tag="oc")
                    for i in range(TPG):
                        t = g * TPG + i
                        half = t // (NT // 2)
                        bc = bcpool.tile([128, CH], f32, tag="bc")
                        nc.tensor.matmul(
                            bc[:],
                            sel[:, t * 128:(t + 1) * 128],
                            xT[half][:, c * CH:(c + 1) * CH],
                            start=True, stop=True,
                        )
                        h = hpool.tile([128, CH], bf16, tag="h")
                        # smooth AAD interleave (runs of same-engine pass1
                        # serialize); t%21 term shifts ~18 tiles to DVE for
                        # engine balance.
                        route_act = (t % 3 != 2) and (t % 21 != 0)
                        if route_act:
                            nc.scalar.activation(
                                h[:], bc[:], mybir.ActivationFunctionType.Relu,
                                bias=biasT[:, t:t + 1], scale=scaleT[:, t:t + 1],
                            )
                        else:
                            # same affine as the ACT route (no relu; pass2
                            # clamps both sides)
                            nc.vector.tensor_scalar(
                                h[:], bc[:],
                                scaleT[:, t:t + 1], biasT[:, t:t + 1],
                                mybir.AluOpType.mult, mybir.AluOpType.add,
                            )
                        # positive convention: h2 = clamp(scaled basis, 0, 1).
                        # const-only operands keep DVE in its fastest mode.
                        h2 = h2pool.tile([128, CH], bf16, tag="h2")
                        nc.vector.tensor_scalar(
                            h2[:], h[:], 1.0, 0.0,
                            mybir.AluOpType.min, mybir.AluOpType.max,
                        )
                        nc.tensor.matmul(
                            oc[:],
                            wpack[:, t * 128:(t + 1) * 128],
                            h2[:],
                            start=(i == 0), stop=(i == TPG - 1),
                        )
                    osb = opool.tile([128, CH], f32, tag="osb")
                    if g % 4 == 3:
                        nc.vector.tensor_scalar(
                            osb[:], oc[:], obias[:, g:g + 1], 0.0,
                            mybir.AluOpType.add, mybir.AluOpType.bypass,
                        )
                    else:
                        nc.scalar.activation(
                            osb[:], oc[:], mybir.ActivationFunctionType.Identity,
                            bias=obias[:, g:g + 1],
                        )
                    nc.sync.dma_start(
                        out=out_ext[g * 128:(g + 1) * 128, c * CH:(c + 1) * CH],
                        in_=osb[:])

    nc.compile()
    return nc


def host_constants(edges, width, W, b):
    """Packed constants. edges/width [F,B], W [F,B,D], b [F,D].

    Row r = f*48 + k: k=0 bin0 special (basis relu(c1-x)/16, weight
    -W0*winv0*16, constant W0 folded into obias), k=1..46 middle bins
    (basis clamp((x-e)/w, 0, 1), weight +W), k=47 bin47 special (basis
    relu(x-e47)/16, weight +W47*winv47*16).
    """
    f32 = np.float32
    winv = (1.0 / width).astype(f32)
    c1 = (edges[:, 0] + width[:, 0]).astype(f32)

    sel = np.zeros((128, NT * 128), np.float16)
    scaleT = np.zeros((128, NT), f32)
    biasT = np.zeros((128, NT), f32)
    eT = np.zeros((128, NT), f32)
    wvD = np.zeros((128, NT), f32)
    wpack = np.zeros((128, NT * 128), f32)
    obias = np.zeros((128, NGRP), f32)

    for r in range(F * B):
        f, k = r // B, r % B
        t, j = r // 128, r % 128          # tile, row-in-tile
        g = f // 16                        # output group
        m = (f % 16) * 8                   # output row base within group
        sel[f % 128, t * 128 + j] = -1.0 if k == 0 else 1.0
        if k == 0:
            scaleT[j, t] = 1.0 / 16.0
            biasT[j, t] = c1[f] / 16.0
            eT[j, t] = -c1[f]
            wvD[j, t] = 1.0 / 16.0
            wpack[j, t * 128 + m:t * 128 + m + 8] = -W[f, 0, :] * winv[f, 0] * 16.0
        elif k == 47:
            scaleT[j, t] = 1.0 / 16.0
            biasT[j, t] = -edges[f, 47] / 16.0
            eT[j, t] = edges[f, 47]
            wvD[j, t] = 1.0 / 16.0
            wpack[j, t * 128 + m:t * 128 + m + 8] = W[f, 47, :] * winv[f, 47] * 16.0
        else:
            scaleT[j, t] = winv[f, k]
            biasT[j, t] = -edges[f, k] * winv[f, k]
            eT[j, t] = edges[f, k]
            wvD[j, t] = winv[f, k]
            wpack[j, t * 128 + m:t * 128 + m + 8] = W[f, k, :]
        assert t // TPG == g

    for f in range(F):
        g, m = f // 16, (f % 16) * 8
        obias[m:m + 8, g] = b[f, :] + W[f, 0, :]

    return {
        "sel": sel,
        "wpack": wpack.astype(ml_dtypes.bfloat16),
        "scaleT": scaleT,
        "biasT": biasT,
        "eT": eT,
        "wvD": wvD,
        "obias": obias,
    }


def make_in_maps(x, edges, width, W, b):
    consts = host_constants(np.asarray(edges), np.asarray(width),
                            np.asarray(W), np.asarray(b))
    x = np.asarray(x, dtype=np.float32)
    in_maps = []
    for core in range(NCORES):
        m = dict(consts)
        m["xT"] = np.ascontiguousarray(x[core * NS:(core + 1) * NS, :].T)
        in_maps.append(m)
    return in_maps


def kernel(x, edges, width, W, b):
    if "nc" not in _cache:
        _cache["nc"] = build_nc()
    nc = _cache["nc"]
    in_maps = make_in_maps(x, edges, width, W, b)
    res = run_bass_kernel_spmd(nc, in_maps, core_ids=list(range(NCORES)))
    outs = [np.ascontiguousarray(np.asarray(r["out"]).T) for r in res.results]
    return np.concatenate(outs, axis=0)


# revision 24
# speedup vs baseline: 1.0074x; 1.0074x over previous
"""Trainium2 Bass kernel for CompactPiecewiseLinearEmbeddings.

out[n, f*8+d] = sum_b h[n,f,b] * W[f,b,d] + b[f,d]
h = piecewise-linear encoding of x[n,f] over per-feature bins
    (first bin clamp_max(1), middle clamp(0,1), last clamp_min(0)).

Strategy (per core; data-parallel over N across 8 cores):
 - Host passes x pre-transposed (xT [F, NS]) and gathers the
   [F*D, NS]-major output with a host-side transpose (both are part of
   shard/unshard).
 - Uniform row space r = f*48 + k over all features/bins (k=0 is the
   bin0 special row, k=1..46 middle bins, k=47 the bin47 special row):
   96 tiles of 128 rows per n-chunk; 16-feature output group = exactly
   6 tiles.
 - Per tile: PE broadcast matmul (signed selector) puts +-x on the 128
   rows; pass1 builds the relu basis (ACT relu(scale*bc+bias), or DVE
   subtract/max, interleaved 2:1 for balance); pass2 clamps to [0, 1]
   on DVE (const-only min for the ACT route, mult/min for the DVE
   route); PE block matmul accumulates 6 tiles into the group's
   [128, CH] PSUM output.
 - ACT or DVE evacuates + adds bias; DMA straight to HBM in [fd, n]
   layout.
"""
import numpy as np
import ml_dtypes

from concourse import bacc, mybir
from concourse.tile import TileContext
from concourse.bass_utils import run_bass_kernel_spmd

N, F, B, D = 16384, 256, 48, 8
NCORES = 8
NS = N // NCORES          # 2048 rows per core
CH = 512                  # n-chunk (PSUM bank = 512 f32 out cols per MM)
NCH = NS // CH            # 4
NGRP = F // 16            # 16 groups of 16 features
NT = F * B // 128         # 96 row-tiles
TPG = NT // NGRP          # 6 tiles per group

_cache = {}


def build_nc():
    nc = bacc.Bacc("TRN2")
    f32, f16, bf16 = mybir.dt.float32, mybir.dt.float16, mybir.dt.bfloat16

    xT_ext = nc.declare_dram_parameter("xT", [F, NS], f32, isOutput=False)
    sel_ext = nc.declare_dram_parameter("sel", [128, NT * 128], f16, isOutput=False)
    wpack_ext = nc.declare_dram_parameter("wpack", [128, NT * 128], bf16, isOutput=False)
    scaleT_ext = nc.declare_dram_parameter("scaleT", [128, NT], f32, isOutput=False)
    biasT_ext = nc.declare_dram_parameter("biasT", [128, NT], f32, isOutput=False)
    eT_ext = nc.declare_dram_parameter("eT", [128, NT], f32, isOutput=False)
    wvD_ext = nc.declare_dram_parameter("wvD", [128, NT], f32, isOutput=False)
    obias_ext = nc.declare_dram_parameter("obias", [128, NGRP], f32, isOutput=False)
    out_ext = nc.declare_dram_parameter("out", [F * D, NS], f32, isOutput=True)

    with TileContext(nc) as tc:
        with (
            tc.tile_pool(name="const", bufs=1) as cpool,
            tc.tile_pool(name="hbuf", bufs=6) as hpool,
            tc.tile_pool(name="h2buf", bufs=6) as h2pool,
            tc.tile_pool(name="osb", bufs=4) as opool,
            tc.tile_pool(name="bc", bufs=4, space="PSUM") as bcpool,
            tc.tile_pool(name="oc", bufs=3, space="PSUM") as ocpool,
        ):
            # ---- load constants ----
            sel = cpool.tile([128, NT * 128], f16)
            wpack = cpool.tile([128, NT * 128], bf16)
            scaleT = cpool.tile([128, NT], f32)
            biasT = cpool.tile([128, NT], f32)
            eT = cpool.tile([128, NT], f32)
            wvD = cpool.tile([128, NT], f32)
            obias = cpool.tile([128, NGRP], f32)
            for t, e in [(scaleT, scaleT_ext), (biasT, biasT_ext), (eT, eT_ext),
                         (wvD, wvD_ext), (obias, obias_ext)]:
                nc.sync.dma_start(out=t[:], in_=e[:])
            # split the two big constant loads so early tiles can start
            # before the full 6MB lands
            for q in range(8):
                s0, s1 = q * NT * 16, (q + 1) * NT * 16
                nc.sync.dma_start(out=sel[:, s0:s1], in_=sel_ext[:, s0:s1])
                nc.sync.dma_start(out=wpack[:, s0:s1], in_=wpack_ext[:, s0:s1])

            # ---- load xT (f32) and cast to f16 rhs tiles ----
            xsb = cpool.tile([128, 2 * NS], f32)
            for ft in range(2):
                nc.sync.dma_start(out=xsb[:, ft * NS:(ft + 1) * NS],
                                  in_=xT_ext[ft * 128:(ft + 1) * 128, :])
            xT = [cpool.tile([128, NS], f16, tag=f"xT{t}", name=f"xT{t}")
                  for t in range(2)]
            for ft in range(2):
                nc.scalar.activation(xT[ft][:], xsb[:, ft * NS:(ft + 1) * NS],
                                     mybir.ActivationFunctionType.Copy)

            # ---- main loop ----
            for c in range(NCH):
                for g in range(NGRP):
                    oc = ocpool.tile([128, CH], f32, tag="oc")
                    for i in range(TPG):
                        t = g * TPG + i
                        half = t // (NT // 2)
                        bc = bcpool.tile([128, CH], f32, tag="bc")
                        nc.tensor.matmul(
                            bc[:],
                            sel[:, t * 128:(t + 1) * 128],
                            xT[half][:, c * CH:(c + 1) * CH],
                            start=True, stop=True,
                        )
                        h = hpool.tile([128, CH], bf16, tag="h")
                        # smooth 5/8 interleave, runs <= 2 (long same-engine
                        # pass1 runs serialize the pipeline)
                        route_act = (t % 8) in (0, 1, 3, 4, 6)
                        if route_act:
                            nc.scalar.activation(
                                h[:], bc[:], mybir.ActivationFunctionType.Relu,
                                bias=biasT[:, t:t + 1], scale=scaleT[:, t:t + 1],
                            )
                        else:
                            # same affine as the ACT route (no relu; pass2
                            # clamps both sides)
                            nc.vector.tensor_scalar(
                                h[:], bc[:],
                                scaleT[:, t:t + 1], biasT[:, t:t + 1],
                                mybir.AluOpType.mult, mybir.AluOpType.add,
                            )
                        # positive convention: h2 = clamp(scaled basis, 0, 1).
                        # const-only operands keep DVE in its fastest mode.
                        h2 = h2pool.tile([128, CH], bf16, tag="h2")
                        nc.vector.tensor_scalar(
                            h2[:], h[:], 1.0, 0.0,
                            mybir.AluOpType.min, mybir.AluOpType.max,
                        )
                        nc.tensor.matmul(
                            oc[:],
                            wpack[:, t * 128:(t + 1) * 128],
                            h2[:],
                            start=(i == 0), stop=(i == TPG - 1),
                        )
                    osb = opool.tile([128, CH], f32, tag="osb")
                    if g % 4 == 3:
                        nc.vector.tensor_scalar(
                            osb[:], oc[:], obias[:, g:g + 1], 0.0,
                            mybir.AluOpType.add, mybir.AluOpType.bypass,
                        )
                    else:
                        nc.scalar.activation(
                            osb[:], oc[:], mybir.ActivationFunctionType.Identity,
                            bias=obias[:, g:g + 1],
                        )
                    nc.sync.dma_start(
                        out=out_ext[g * 128:(g + 1) * 128, c * CH:(c + 1) * CH],
                        in_=osb[:])

    nc.compile()
    return nc


def host_constants(edges, width, W, b):
    """Packed constants. edges/width [F,B], W [F,B,D], b [F,D].

    Row r = f*48 + k: k=0 bin0 special (basis relu(c1-x)/16, weight
    -W0*winv0*16, constant W0 folded into obias), k=1..46 middle bins
    (basis clamp((x-e)/w, 0, 1), weight +W), k=47 bin47 special (basis
    relu(x-e47)/16, weight +W47*winv47*16).
    """
    f32 = np.float32
    winv = (1.0 / width).astype(f32)
    c1 = (edges[:, 0] + width[:, 0]).astype(f32)

    sel = np.zeros((128, NT * 128), np.float16)
    scaleT = np.zeros((128, NT), f32)
    biasT = np.zeros((128, NT), f32)
    eT = np.zeros((128, NT), f32)
    wvD = np.zeros((128, NT), f32)
    wpack = np.zeros((128, NT * 128), f32)
    obias = np.zeros((128, NGRP), f32)

    for r in range(F * B):
        f, k = r // B, r % B
        t, j = r // 128, r % 128          # tile, row-in-tile
        g = f // 16                        # output group
        m = (f % 16) * 8                   # output row base within group
        sel[f % 128, t * 128 + j] = -1.0 if k == 0 else 1.0
        if k == 0:
            scaleT[j, t] = 1.0 / 16.0
            biasT[j, t] = c1[f] / 16.0
            eT[j, t] = -c1[f]
            wvD[j, t] = 1.0 / 16.0
            wpack[j, t * 128 + m:t * 128 + m + 8] = -W[f, 0, :] * winv[f, 0] * 16.0
        elif k == 47:
            scaleT[j, t] = 1.0 / 16.0
            biasT[j, t] = -edges[f, 47] / 16.0
            eT[j, t] = edges[f, 47]
            wvD[j, t] = 1.0 / 16.0
            wpack[j, t * 128 + m:t * 128 + m + 8] = W[f, 47, :] * winv[f, 47] * 16.0
        else:
            scaleT[j, t] = winv[f, k]
            biasT[j, t] = -edges[f, k] * winv[f, k]
            eT[j, t] = edges[f, k]
            wvD[j, t] = winv[f, k]
            wpack[j, t * 128 + m:t * 128 + m + 8] = W[f, k, :]
        assert t // TPG == g

    for f in range(F):
        g, m = f // 16, (f % 16) * 8
        obias[m:m + 8, g] = b[f, :] + W[f, 0, :]

    return {
        "sel": sel,
        "wpack": wpack.astype(ml_dtypes.bfloat16),
        "scaleT": scaleT,
        "biasT": biasT,
        "eT": eT,
        "wvD": wvD,
        "obias": obias,
    }


def make_in_maps(x, edges, width, W, b):
    consts = host_constants(np.asarray(edges), np.asarray(width),
                            np.asarray(W), np.asarray(b))
    x = np.asarray(x, dtype=np.float32)
    in_maps = []
    for core in range(NCORES):
        m = dict(consts)
        m["xT"] = np.ascontiguousarray(x[core * NS:(core + 1) * NS, :].T)
        in_maps.append(m)
    return in_maps


def kernel(x, edges, width, W, b):
    if "nc" not in _cache:
        _cache["nc"] = build_nc()
    nc = _cache["nc"]
    in_maps = make_in_maps(x, edges, width, W, b)
    res = run_bass_kernel_spmd(nc, in_maps, core_ids=list(range(NCORES)))
    outs = [np.ascontiguousarray(np.asarray(r["out"]).T) for r in res.results]
    return np.concatenate(outs, axis=0)
